# revision 1
# baseline (speedup 1.0000x reference)
"""R-GCN (2-layer basis-decomposition GCN) on 8 Trainium2 NeuronCores.

Strategy (1D node partition, per sharding hint):
- Nodes sharded 1024/core. Each core computes its support rows
  sup1 = feat_shard @ V1cat via PE-transpose + fp32 matmul, AllGathers the
  full [8192,256] table to Shared DRAM.
- Edges sharded by destination node, bucketed per (dst-block of 128, relation),
  padded to 128-edge chunks (pad: src=0, w=0).
- Messages gathered with gpsimd.dma_gather (256B rows) landing as
  [128 edges (partitions), 64 feats] — directly the matmul moving operand.
- segment_sum = one-hot matmul: stationary [128e,128d] weighted one-hot built
  by one DVE tensor_scalar (iota is_equal dst) * w; PSUM accumulates per block.
- Layer 2 identical with a [8192,192] padded table; classifier on PE.
- Wc1/Wc2/bclf are baked into the program as immediates (program is built per
  call); the basis combination V = Wc x W itself is computed on-device (DVE).
"""
import os
import sys
import numpy as np

sys.path.insert(0, "/opt/trn_rl_repo")
from concourse import bacc, bass, mybir, tile  # noqa: E402
from concourse.bass_utils import run_bass_kernel_spmd  # noqa: E402

F32 = mybir.dt.float32
F32R = mybir.dt.float32r
I16 = mybir.dt.int16
I32 = mybir.dt.int32

N = 8192
S = 4
E = 262144
H = 64
F = 32
C = 2
NCORES = 8
NPC = N // NCORES      # 1024 nodes per core
NB = NPC // 128        # 8 dst blocks per core
KCH = N // 128         # 64 contraction chunks for layer 1
T2COLS = 192           # layer-2 table padded cols (768B rows)

# f32r moving operand for the big support matmul (4x PE speedup, measured
# ~1e-3 rel err); flip to False for full fp32.
USE_F32R = True


VDT = F32R if USE_F32R else F32


def _mm(nc, out, lhsT, rhs, **kw):
    kw.pop("f32r", None)
    nc.tensor.matmul(out, lhsT=lhsT, rhs=rhs, **kw)


PHASES = int(os.environ.get("_GCN87_PHASES_DEBUG", "5"))


def build_program(cnt, wc1, wc2, bclf_v):
    """cnt: [NB][S] padded edge counts (identical across cores)."""
    nc = bacc.Bacc(None)
    ncs = nc  # alias

    feat = nc.dram_tensor("feat", [NPC, N], F32, kind="ExternalInput")
    w1 = nc.dram_tensor("w1", [2, N, H], F32, kind="ExternalInput")
    w2 = nc.dram_tensor("w2", [2, H, F], F32, kind="ExternalInput")
    wclf = nc.dram_tensor("wclf", [F, C], F32, kind="ExternalInput")
    bc = nc.dram_tensor("bc", [C, 1], F32, kind="ExternalInput")
    tot = sum(cnt[b][s] for b in range(NB) for s in range(S))
    eidx = nc.dram_tensor("eidx", [128, tot // 16], I16, kind="ExternalInput")
    emeta = nc.dram_tensor("emeta", [128, 2 * (tot // 128)], F32, kind="ExternalInput")
    out = nc.dram_tensor("out", [C, NPC], F32, kind="ExternalOutput")

    ag1_in = nc.dram_tensor("ag1_in", [NPC, 4 * H], F32)
    table1 = nc.dram_tensor("table1", [N, 4 * H], F32, addr_space="Shared")
    ag2_in = nc.dram_tensor("ag2_in", [NPC, T2COLS], F32)
    table2 = nc.dram_tensor("table2", [N, T2COLS], F32, addr_space="Shared")

    rg = [list(range(NCORES))]
    nch_max = max(cnt[b][s] for b in range(NB) for s in range(S)) // 128
    ncol = tot // 128  # emeta columns per half

    with tile.TileContext(nc) as tc:
        with tc.tile_pool(name="const", bufs=1) as cp:
            # ---- constants ----
            iota_i = cp.tile([128, 128], I32)
            nc.gpsimd.iota(iota_i, pattern=[[1, 128]], base=0, channel_multiplier=0)
            iota_f = cp.tile([128, 128], F32)
            nc.vector.tensor_copy(iota_f, iota_i)
            idn_i = cp.tile([128, 128], I32)
            nc.gpsimd.iota(idn_i, pattern=[[1, 128]], base=0, channel_multiplier=-1)
            ident = cp.tile([128, 128], F32)
            nc.vector.tensor_scalar(
                ident, idn_i, 0, None, mybir.AluOpType.is_equal
            )

            eidx_sb = cp.tile([128, tot // 16], I16)
            nc.sync.dma_start(eidx_sb, eidx[:, :])
            emeta_sb = cp.tile([128, 2 * ncol], F32)
            nc.sync.dma_start(emeta_sb, emeta[:, :])
            edst_sb = emeta_sb[:, :ncol]
            ew_sb = emeta_sb[:, ncol:]

            x1_sb = cp.tile([128, NB, H], F32)
            x1t_sb = cp.tile([H, NPC], F32)
            x2_sb = cp.tile([128, NB, F], F32)
            v2_sb = cp.tile([H, 4 * F], F32)
            wclf_sb = cp.tile([F, C], F32)
            nc.sync.dma_start(wclf_sb, wclf[:, :])
            bclf_sb = cp.tile([C, 1], F32)
            nc.sync.dma_start(bclf_sb, bc[:, :])
            out_sb = cp.tile([C, NPC], F32)

            # ---- phase 1: V1cat build + support matmul ----
            with (
                tc.tile_pool(name="ph1", bufs=2) as p1,
                tc.tile_pool(name="v1p", bufs=1) as v1p,
                tc.tile_pool(name="fpc", bufs=4) as fpc,
                tc.tile_pool(name="ph1ps", bufs=2, space="PSUM") as p1ps,
                tc.tile_pool(name="ptps", bufs=3, space="PSUM") as ptps,
            ):
                v1 = v1p.tile([128, KCH, 4 * H], VDT)
                for k in range(KCH):
                    ksl = slice(128 * k, 128 * (k + 1))
                    w1a = p1.tile([128, H], F32, tag="w1a")
                    nc.sync.dma_start(w1a, w1[0, ksl, :])
                    w1b = p1.tile([128, H], F32, tag="w1b")
                    nc.sync.dma_start(w1b, w1[1, ksl, :])
                    for s in range(S):
                        tmp = p1.tile([128, H], F32, tag="vtmp")
                        nc.vector.tensor_scalar(
                            tmp, w1b, float(wc1[s, 1]), None, mybir.AluOpType.mult
                        )
                        nc.vector.scalar_tensor_tensor(
                            v1[:, k, H * s : H * (s + 1)],
                            w1a,
                            float(wc1[s, 0]),
                            tmp,
                            mybir.AluOpType.mult,
                            mybir.AluOpType.add,
                        )

                for nb in range(NB):
                    nsl = slice(128 * nb, 128 * (nb + 1))
                    ps_sup = p1ps.tile([128, 4 * H], F32, tag="pssup")
                    for kk in range(KCH // 4):
                        piece = fpc.tile([128, 512], F32, tag="piece")
                        nc.sync.dma_start(
                            piece, feat[nsl, 512 * kk : 512 * (kk + 1)]
                        )
                        for j in range(4):
                            k = 4 * kk + j
                            pt = ptps.tile([128, 128], F32, tag="pt")
                            nc.tensor.transpose(
                                pt, piece[:, 128 * j : 128 * (j + 1)], ident
                            )
                            ft = fpc.tile([128, 128], VDT, tag="ft")
                            nc.vector.tensor_copy(ft, pt)
                            _mm(
                                nc, ps_sup, ft, v1[:, k, :],
                                start=(k == 0), stop=(k == KCH - 1), f32r=True,
                            )
                    sup_sb = p1.tile([128, 4 * H], F32, tag="supsb")
                    nc.any.tensor_copy(sup_sb, ps_sup)
                    nc.sync.dma_start(ag1_in[nsl, :], sup_sb)

            if PHASES >= 1:
                nc.gpsimd.collective_compute(
                    "AllGather", mybir.AluOpType.bypass, replica_groups=rg,
                    ins=[ag1_in[:]], outs=[table1[:]],
                )

            # ---- layer-1 aggregation ----
            def agg_layer(gbp, ohp, aps, table, col_off_mul, col_step, nfeat, dst_sb, layer):
                off = 0
                for nb in range(NB):
                    psx = aps.tile([128, nfeat], F32, tag=f"psx{layer}")
                    nmm = sum(cnt[nb][s] // 128 for s in range(S))
                    mi = 0
                    for s in range(S):
                        cn = cnt[nb][s]
                        done = 0
                        while done < cn:
                            sub = min(1024, cn - done)
                            nch = sub // 128
                            gb = gbp.tile([128, 8, 64], F32, tag="gb")
                            nc.gpsimd.dma_gather(
                                gb[:, :nch, :],
                                table[:, col_off_mul * s : col_off_mul * s + 64],
                                eidx_sb[:, (off + done) // 16 : (off + done + sub) // 16],
                                num_idxs=sub,
                                num_idxs_reg=sub,
                                elem_size=64,
                                elem_step=col_step,
                            )
                            for ch in range(nch):
                                col = (off + done) // 128 + ch
                                oh = ohp.tile([128, 128], F32, tag="oh")
                                nc.vector.tensor_scalar(
                                    oh, iota_f,
                                    edst_sb[:, col : col + 1],
                                    ew_sb[:, col : col + 1],
                                    mybir.AluOpType.is_equal,
                                    mybir.AluOpType.mult,
                                )
                                nc.tensor.matmul(
                                    psx, lhsT=oh, rhs=gb[:, ch, :nfeat],
                                    start=(mi == 0), stop=(mi == nmm - 1),
                                )
                                mi += 1
                            done += sub
                        off += cn
                    nc.scalar.activation(
                        dst_sb[:, nb, :], psx, mybir.ActivationFunctionType.Tanh
                    )

            with (
                tc.tile_pool(name="gbp", bufs=8) as gbp,
                tc.tile_pool(name="ohp", bufs=8) as ohp,
            ):
                if PHASES >= 2:
                    with tc.tile_pool(name="aps1", bufs=2, space="PSUM") as aps1:
                        agg_layer(gbp, ohp, aps1, table1, H, 4 * H, H, x1_sb, 1)

                # ---- layer-2 supports ----
                for s in range(S):
                    w2a = gbp.tile([H, F], F32, tag="w2a")
                    nc.sync.dma_start(w2a, w2[0, :, :])
                    w2b = gbp.tile([H, F], F32, tag="w2b")
                    nc.sync.dma_start(w2b, w2[1, :, :])
                    tmp2 = gbp.tile([H, F], F32, tag="vtmp2")
                    nc.vector.tensor_scalar(
                        tmp2, w2b, float(wc2[s, 1]), None, mybir.AluOpType.mult
                    )
                    nc.vector.scalar_tensor_tensor(
                        v2_sb[:, F * s : F * (s + 1)], w2a, float(wc2[s, 0]),
                        tmp2, mybir.AluOpType.mult, mybir.AluOpType.add,
                    )
                if PHASES >= 3:
                  with tc.tile_pool(name="s2ps", bufs=2, space="PSUM") as s2ps:
                    for nb in range(NB):
                        nsl = slice(128 * nb, 128 * (nb + 1))
                        ptx = s2ps.tile([H, 128], F32, tag="ptx")
                        nc.tensor.transpose(ptx, x1_sb[:, nb, :], ident)
                        nc.any.tensor_copy(x1t_sb[:, nsl], ptx)
                        ps2 = s2ps.tile([128, 4 * F], F32, tag="ps2")
                        nc.tensor.matmul(
                            ps2, lhsT=x1t_sb[:, nsl], rhs=v2_sb, start=True, stop=True
                        )
                        s2_sb = gbp.tile([128, 4 * F], F32, tag="s2sb")
                        nc.any.tensor_copy(s2_sb, ps2)
                        nc.sync.dma_start(ag2_in[nsl, : 4 * F], s2_sb)

                if PHASES >= 3:
                    nc.gpsimd.collective_compute(
                        "AllGather", mybir.AluOpType.bypass, replica_groups=rg,
                        ins=[ag2_in[:]], outs=[table2[:]],
                    )

                # ---- layer-2 aggregation ----
                if PHASES >= 4:
                    with tc.tile_pool(name="aps2", bufs=2, space="PSUM") as aps2:
                        agg_layer(gbp, ohp, aps2, table2, F, T2COLS, F, x2_sb, 2)

                # ---- classifier ----
                if PHASES < 5:
                    nc.vector.memset(out_sb, 0.0)
                with tc.tile_pool(name="clfps", bufs=2, space="PSUM") as clfps:
                    for nb in (range(NB) if PHASES >= 5 else []):
                        nsl = slice(128 * nb, 128 * (nb + 1))
                        ptc = clfps.tile([F, 128], F32, tag="ptc")
                        nc.tensor.transpose(ptc, x2_sb[:, nb, :], ident)
                        x2t = gbp.tile([F, 128], F32, tag="x2t")
                        nc.any.tensor_copy(x2t, ptc)
                        pso = clfps.tile([C, 128], F32, tag="pso")
                        nc.tensor.matmul(pso, lhsT=wclf_sb, rhs=x2t, start=True, stop=True)
                        nc.vector.tensor_scalar(
                            out_sb[:, nsl], pso, bclf_sb[:, 0:1], None,
                            mybir.AluOpType.add,
                        )
                nc.sync.dma_start(out[:, :], out_sb)
    nc.finalize()
    return nc


def _prep_edges(edge_src, edge_dst, edge_w):
    """Bucket edges per (core, block, relation); pad to uniform chunk counts."""
    buckets = [[[None] * S for _ in range(NB)] for _ in range(NCORES)]
    for s in range(S):
        dst = edge_dst[s]
        core = dst // NPC
        blk = (dst % NPC) // 128
        dloc = dst % 128
        for c in range(NCORES):
            mc = core == c
            for b in range(NB):
                m = mc & (blk == b)
                buckets[c][b][s] = (
                    edge_src[s][m], dloc[m], edge_w[s][m]
                )
    cnt = [
        [
            ((max(len(buckets[c][b][s][0]) for c in range(NCORES)) + 127) // 128)
            * 128
            for s in range(S)
        ]
        for b in range(NB)
    ]
    tot = sum(cnt[b][s] for b in range(NB) for s in range(S))

    eidx_all, emeta_all = [], []
    for c in range(NCORES):
        src_st = np.zeros(tot, np.int16)
        dst_st = np.zeros(tot, np.float32)
        w_st = np.zeros(tot, np.float32)
        off = 0
        for b in range(NB):
            for s in range(S):
                sr, dl, w = buckets[c][b][s]
                n = len(sr)
                src_st[off : off + n] = sr.astype(np.int16)
                dst_st[off : off + n] = dl.astype(np.float32)
                w_st[off : off + n] = w
                off += cnt[b][s]
        eidx = np.tile(src_st.reshape(tot // 16, 16).T, (8, 1)).copy()
        edst = dst_st.reshape(tot // 128, 128).T
        ew = w_st.reshape(tot // 128, 128).T
        eidx_all.append(np.ascontiguousarray(eidx))
        emeta_all.append(np.ascontiguousarray(np.concatenate([edst, ew], axis=1)))
    return cnt, eidx_all, emeta_all


def kernel(features, edge_w, W1, Wc1, W2, Wc2, Wclf, bclf, edge_src, edge_dst):
    features = np.asarray(features, np.float32)
    edge_w = np.asarray(edge_w, np.float32)
    W1 = np.asarray(W1, np.float32)
    Wc1 = np.asarray(Wc1, np.float32)
    W2 = np.asarray(W2, np.float32)
    Wc2 = np.asarray(Wc2, np.float32)
    Wclf = np.asarray(Wclf, np.float32)
    bclf = np.asarray(bclf, np.float32)
    edge_src = np.asarray(edge_src, np.int32)
    edge_dst = np.asarray(edge_dst, np.int32)

    cnt, eidx_all, emeta_all = _prep_edges(edge_src, edge_dst, edge_w)
    nc = build_program(cnt, Wc1, Wc2, bclf)

    in_maps = [
        dict(
            feat=np.ascontiguousarray(features[c * NPC : (c + 1) * NPC]),
            w1=W1, w2=W2, wclf=Wclf, bc=bclf.reshape(C, 1),
            eidx=eidx_all[c], emeta=emeta_all[c],
        )
        for c in range(NCORES)
    ]
    res = run_bass_kernel_spmd(nc, in_maps, list(range(NCORES))).results
    return np.concatenate([res[c]["out"].T for c in range(NCORES)], axis=0)



# revision 4
# speedup vs baseline: 2.2671x; 2.2671x over previous
"""R-GCN (2-layer basis-decomposition GCN) on 8 Trainium2 NeuronCores.

Strategy (1D node partition, per sharding hint):
- Nodes sharded 1024/core. Host sends each core its feature shard
  TRANSPOSED and in bf16 ([8192, 1024], halves the dominant host->device
  transfer and removes on-device PE transposes).
- The small basis combinations V1 = Wc1 x W1 ([8192, 256]) and
  V2 = Wc2 x W2 ([64, 128]) are computed on host. V1 is row-sharded
  (128KB/core bf16) and AllGathered on device; V2 is replicated (32KB).
- Each core computes sup1 = feat_shard @ V1cat with bf16 matmuls (PSUM
  f32 accumulate), AllGathers the full [8192, 256] f32 table to Shared
  DRAM.
- Edges sharded by destination node, bucketed per (dst-block of 128,
  relation), padded to 128-edge chunks (pad: src=0, w=0). Edge index
  stream is sent untiled ([16, tot/16] int16) and replicated to the 128
  partitions on device; local dst rows as uint8; weights f32.
- Messages gathered with gpsimd.dma_gather (256B rows) landing as
  [128 edges (partitions), 64 feats] — directly the matmul moving operand.
- segment_sum = one-hot matmul: stationary [128e,128d] weighted one-hot
  built by one DVE tensor_scalar (iota is_equal dst) * w; PSUM
  accumulates per block.
- Layer 2 identical with a [8192,192] padded table; classifier on PE.
"""
import sys
import numpy as np
import ml_dtypes

sys.path.insert(0, "/opt/trn_rl_repo")
from concourse import bacc, bass, mybir, tile  # noqa: E402
from concourse.bass_utils import run_bass_kernel_spmd  # noqa: E402

BF16 = mybir.dt.bfloat16
F32 = mybir.dt.float32
I16 = mybir.dt.int16
I32 = mybir.dt.int32
U8 = mybir.dt.uint8
NPBF16 = ml_dtypes.bfloat16

N = 8192
S = 4
E = 262144
H = 64
F = 32
C = 2
NCORES = 8
NPC = N // NCORES      # 1024 nodes per core
NB = NPC // 128        # 8 dst blocks per core
KCH = N // 128         # 64 contraction chunks for layer 1
T2COLS = 192           # layer-2 table padded cols (768B rows)


def build_program(cnt):
    """cnt: [NB][S] padded edge counts (identical across cores)."""
    nc = bacc.Bacc(None)

    tot = sum(cnt[b][s] for b in range(NB) for s in range(S))
    ncol = tot // 128

    featT = nc.dram_tensor("featT", [N, NPC], BF16, kind="ExternalInput")
    v1s = nc.dram_tensor("v1s", [NPC, 4 * H], BF16, kind="ExternalInput")
    v2c = nc.dram_tensor("v2c", [H, 4 * F], F32, kind="ExternalInput")
    wclf = nc.dram_tensor("wclf", [F, C], F32, kind="ExternalInput")
    bc = nc.dram_tensor("bc", [C, 1], F32, kind="ExternalInput")
    eidx = nc.dram_tensor("eidx", [16, tot // 16], I16, kind="ExternalInput")
    edst8 = nc.dram_tensor("edst8", [128, ncol], U8, kind="ExternalInput")
    ew = nc.dram_tensor("ew", [128, ncol], F32, kind="ExternalInput")
    out = nc.dram_tensor("out", [C, NPC], F32, kind="ExternalOutput")

    agv1 = nc.dram_tensor("agv1", [NPC, 4 * H], BF16)
    tbv1 = nc.dram_tensor("tbv1", [N, 4 * H], BF16, addr_space="Shared")
    ag1_in = nc.dram_tensor("ag1_in", [NPC, 4 * H], F32)
    table1 = nc.dram_tensor("table1", [N, 4 * H], F32, addr_space="Shared")
    ag2_in = nc.dram_tensor("ag2_in", [NPC, T2COLS], F32)
    table2 = nc.dram_tensor("table2", [N, T2COLS], F32, addr_space="Shared")

    rg = [list(range(NCORES))]

    with tile.TileContext(nc) as tc:
        with tc.tile_pool(name="const", bufs=1) as cp:
            # ---- constants ----
            iota_i = cp.tile([128, 128], I32)
            nc.gpsimd.iota(iota_i, pattern=[[1, 128]], base=0, channel_multiplier=0)
            iota_f = cp.tile([128, 128], F32)
            nc.vector.tensor_copy(iota_f, iota_i)
            idn_i = cp.tile([128, 128], I32)
            nc.gpsimd.iota(idn_i, pattern=[[1, 128]], base=0, channel_multiplier=-1)
            ident = cp.tile([128, 128], F32)
            nc.vector.tensor_scalar(
                ident, idn_i, 0, None, mybir.AluOpType.is_equal
            )

            # edge streams: replicate idx block to all 8 gpsimd stripes
            eidx_sb = cp.tile([128, tot // 16], I16)
            for i in range(8):
                nc.sync.dma_start(eidx_sb[16 * i : 16 * (i + 1), :], eidx[:, :])
            edst8_sb = cp.tile([128, ncol], U8)
            nc.sync.dma_start(edst8_sb, edst8[:, :])
            edst_sb = cp.tile([128, ncol], F32)
            nc.vector.tensor_copy(edst_sb, edst8_sb)
            ew_sb = cp.tile([128, ncol], F32)
            nc.sync.dma_start(ew_sb, ew[:, :])

            x1_sb = cp.tile([128, NB, H], F32)
            x1t_sb = cp.tile([H, NPC], F32)
            x2_sb = cp.tile([128, NB, F], F32)
            v2_sb = cp.tile([H, 4 * F], F32)
            nc.sync.dma_start(v2_sb, v2c[:, :])
            wclf_sb = cp.tile([F, C], F32)
            nc.sync.dma_start(wclf_sb, wclf[:, :])
            bclf_sb = cp.tile([C, 1], F32)
            nc.sync.dma_start(bclf_sb, bc[:, :])
            out_sb = cp.tile([C, NPC], F32)

            # ---- phase 0: AllGather V1 (host-combined, row-sharded) ----
            v1b = cp.tile([128, NB, 4 * H], BF16)
            for b in range(NB):
                nc.sync.dma_start(v1b[:, b, :], v1s[128 * b : 128 * (b + 1), :])
            for b in range(NB):
                nc.sync.dma_start(agv1[128 * b : 128 * (b + 1), :], v1b[:, b, :])
            nc.gpsimd.collective_compute(
                "AllGather", mybir.AluOpType.bypass, replica_groups=rg,
                ins=[agv1[:]], outs=[tbv1[:]],
            )
            v1 = cp.tile([128, KCH, 4 * H], BF16)
            for k in range(KCH):
                nc.sync.dma_start(v1[:, k, :], tbv1[128 * k : 128 * (k + 1), :])

            # ---- phase 1: support matmul sup1 = featT.T @ V1cat ----
            with (
                tc.tile_pool(name="ftp", bufs=3) as ftp,
                tc.tile_pool(name="spp", bufs=1, space="PSUM") as spp,
                tc.tile_pool(name="ssb", bufs=2) as ssb,
            ):
                ps = [
                    spp.tile([128, 4 * H], F32, tag=f"ps{b}", name=f"ps{b}")
                    for b in range(NB)
                ]
                for k in range(KCH):
                    ftk = ftp.tile([128, NPC], BF16, tag="ftk")
                    nc.sync.dma_start(ftk, featT[128 * k : 128 * (k + 1), :])
                    for b in range(NB):
                        nc.tensor.matmul(
                            ps[b], lhsT=ftk[:, 128 * b : 128 * (b + 1)],
                            rhs=v1[:, k, :],
                            start=(k == 0), stop=(k == KCH - 1),
                        )
                for b in range(NB):
                    s_sb = ssb.tile([128, 4 * H], F32, tag="ssb")
                    nc.any.tensor_copy(s_sb, ps[b])
                    nc.sync.dma_start(ag1_in[128 * b : 128 * (b + 1), :], s_sb)

            nc.gpsimd.collective_compute(
                "AllGather", mybir.AluOpType.bypass, replica_groups=rg,
                ins=[ag1_in[:]], outs=[table1[:]],
            )

            # ---- aggregation (shared by both layers) ----
            def agg_layer(gbp, ohp, aps, table, col_off_mul, col_step, nfeat, dst_sb, layer):
                off = 0
                for nb in range(NB):
                    psx = aps.tile([128, nfeat], F32, tag=f"psx{layer}")
                    nmm = sum(cnt[nb][s] // 128 for s in range(S))
                    mi = 0
                    for s in range(S):
                        cn = cnt[nb][s]
                        done = 0
                        while done < cn:
                            sub = min(1024, cn - done)
                            nch = sub // 128
                            gb = gbp.tile([128, 8, 64], F32, tag="gb")
                            nc.gpsimd.dma_gather(
                                gb[:, :nch, :],
                                table[:, col_off_mul * s : col_off_mul * s + 64],
                                eidx_sb[:, (off + done) // 16 : (off + done + sub) // 16],
                                num_idxs=sub,
                                num_idxs_reg=sub,
                                elem_size=64,
                                elem_step=col_step,
                            )
                            for ch in range(nch):
                                col = (off + done) // 128 + ch
                                oh = ohp.tile([128, 128], F32, tag="oh")
                                nc.vector.tensor_scalar(
                                    oh, iota_f,
                                    edst_sb[:, col : col + 1],
                                    ew_sb[:, col : col + 1],
                                    mybir.AluOpType.is_equal,
                                    mybir.AluOpType.mult,
                                )
                                nc.tensor.matmul(
                                    psx, lhsT=oh, rhs=gb[:, ch, :nfeat],
                                    start=(mi == 0), stop=(mi == nmm - 1),
                                )
                                mi += 1
                            done += sub
                        off += cn
                    nc.scalar.activation(
                        dst_sb[:, nb, :], psx, mybir.ActivationFunctionType.Tanh
                    )

            with (
                tc.tile_pool(name="gbp", bufs=8) as gbp,
                tc.tile_pool(name="ohp", bufs=8) as ohp,
            ):
                with tc.tile_pool(name="aps1", bufs=2, space="PSUM") as aps1:
                    agg_layer(gbp, ohp, aps1, table1, H, 4 * H, H, x1_sb, 1)

                # ---- layer-2 supports (V2 host-combined) ----
                with tc.tile_pool(name="s2ps", bufs=2, space="PSUM") as s2ps:
                    for nb in range(NB):
                        nsl = slice(128 * nb, 128 * (nb + 1))
                        ptx = s2ps.tile([H, 128], F32, tag="ptx")
                        nc.tensor.transpose(ptx, x1_sb[:, nb, :], ident)
                        nc.any.tensor_copy(x1t_sb[:, nsl], ptx)
                        ps2 = s2ps.tile([128, 4 * F], F32, tag="ps2")
                        nc.tensor.matmul(
                            ps2, lhsT=x1t_sb[:, nsl], rhs=v2_sb, start=True, stop=True
                        )
                        s2_sb = gbp.tile([128, 4 * F], F32, tag="s2sb")
                        nc.any.tensor_copy(s2_sb, ps2)
                        nc.sync.dma_start(ag2_in[nsl, : 4 * F], s2_sb)

                nc.gpsimd.collective_compute(
                    "AllGather", mybir.AluOpType.bypass, replica_groups=rg,
                    ins=[ag2_in[:]], outs=[table2[:]],
                )

                # ---- layer-2 aggregation ----
                with tc.tile_pool(name="aps2", bufs=2, space="PSUM") as aps2:
                    agg_layer(gbp, ohp, aps2, table2, F, T2COLS, F, x2_sb, 2)

                # ---- classifier ----
                with tc.tile_pool(name="clfps", bufs=2, space="PSUM") as clfps:
                    for nb in range(NB):
                        nsl = slice(128 * nb, 128 * (nb + 1))
                        ptc = clfps.tile([F, 128], F32, tag="ptc")
                        nc.tensor.transpose(ptc, x2_sb[:, nb, :], ident)
                        x2t = gbp.tile([F, 128], F32, tag="x2t")
                        nc.any.tensor_copy(x2t, ptc)
                        pso = clfps.tile([C, 128], F32, tag="pso")
                        nc.tensor.matmul(pso, lhsT=wclf_sb, rhs=x2t, start=True, stop=True)
                        nc.vector.tensor_scalar(
                            out_sb[:, nsl], pso, bclf_sb[:, 0:1], None,
                            mybir.AluOpType.add,
                        )
                nc.sync.dma_start(out[:, :], out_sb)
    nc.finalize()
    return nc


def _prep_edges(edge_src, edge_dst, edge_w):
    """Bucket edges per (core, block, relation); pad to uniform chunk counts."""
    buckets = [[[None] * S for _ in range(NB)] for _ in range(NCORES)]
    for s in range(S):
        dst = edge_dst[s]
        core = dst // NPC
        blk = (dst % NPC) // 128
        dloc = dst % 128
        for c in range(NCORES):
            mc = core == c
            for b in range(NB):
                m = mc & (blk == b)
                buckets[c][b][s] = (
                    edge_src[s][m], dloc[m], edge_w[s][m]
                )
    cnt = [
        [
            ((max(len(buckets[c][b][s][0]) for c in range(NCORES)) + 127) // 128)
            * 128
            for s in range(S)
        ]
        for b in range(NB)
    ]
    tot = sum(cnt[b][s] for b in range(NB) for s in range(S))

    eidx_all, edst_all, ew_all = [], [], []
    for c in range(NCORES):
        src_st = np.zeros(tot, np.int16)
        dst_st = np.zeros(tot, np.uint8)
        w_st = np.zeros(tot, np.float32)
        off = 0
        for b in range(NB):
            for s in range(S):
                sr, dl, w = buckets[c][b][s]
                n = len(sr)
                src_st[off : off + n] = sr.astype(np.int16)
                dst_st[off : off + n] = dl.astype(np.uint8)
                w_st[off : off + n] = w
                off += cnt[b][s]
        eidx_all.append(np.ascontiguousarray(src_st.reshape(tot // 16, 16).T))
        edst_all.append(np.ascontiguousarray(dst_st.reshape(tot // 128, 128).T))
        ew_all.append(np.ascontiguousarray(w_st.reshape(tot // 128, 128).T))
    return cnt, eidx_all, edst_all, ew_all


def _prep_inputs(features, edge_w, W1, Wc1, W2, Wc2, Wclf, bclf, edge_src, edge_dst):
    """Host prep: bucket edges, combine bases, transpose+bf16 features.
    Returns (cnt, in_maps)."""
    features = np.asarray(features, np.float32)
    edge_w = np.asarray(edge_w, np.float32)
    W1 = np.asarray(W1, np.float32)
    Wc1 = np.asarray(Wc1, np.float32)
    W2 = np.asarray(W2, np.float32)
    Wc2 = np.asarray(Wc2, np.float32)
    Wclf = np.asarray(Wclf, np.float32)
    bclf = np.asarray(bclf, np.float32)
    edge_src = np.asarray(edge_src, np.int32)
    edge_dst = np.asarray(edge_dst, np.int32)

    cnt, eidx_all, edst_all, ew_all = _prep_edges(edge_src, edge_dst, edge_w)

    # features: bf16, transposed, grouped per core so slices are contiguous
    feat_bf = features.astype(NPBF16)
    featT_big = np.ascontiguousarray(
        feat_bf.T.reshape(N, NCORES, NPC).transpose(1, 0, 2)
    ).reshape(NCORES * N, NPC)

    # host-side basis combination (small): V = Wc x W
    V1 = np.einsum("sb,bio->sio", Wc1, W1)              # [S, N, H]
    v1cat = np.concatenate([V1[s] for s in range(S)], axis=1).astype(NPBF16)
    V2 = np.einsum("sb,bio->sio", Wc2, W2)              # [S, H, F]
    v2cat = np.ascontiguousarray(
        np.concatenate([V2[s] for s in range(S)], axis=1).astype(np.float32)
    )

    in_maps = [
        dict(
            featT=featT_big[c * N : (c + 1) * N],
            v1s=v1cat[c * NPC : (c + 1) * NPC],
            v2c=v2cat,
            wclf=Wclf,
            bc=bclf.reshape(C, 1),
            eidx=eidx_all[c],
            edst8=edst_all[c],
            ew=ew_all[c],
        )
        for c in range(NCORES)
    ]
    return cnt, in_maps


def kernel(features, edge_w, W1, Wc1, W2, Wc2, Wclf, bclf, edge_src, edge_dst):
    cnt, in_maps = _prep_inputs(
        features, edge_w, W1, Wc1, W2, Wc2, Wclf, bclf, edge_src, edge_dst
    )
    nc = build_program(cnt)
    res = run_bass_kernel_spmd(nc, in_maps, list(range(NCORES))).results
    return np.concatenate([res[c]["out"].T for c in range(NCORES)], axis=0)
